# revision 1
# baseline (speedup 1.0000x reference)
"""Trainium2 Bass kernel for the CovidModel scenario forecaster.

Math: the reference's 365-day lax.scan linearizes exactly.  With
s(tau) = a0(tau) + eps*a1(tau) (the combined covariate):
    s(tau) = s(tau-1) * K * rt(tau)^(1/T),   K = delta0 + eps*delta1
and the three Poisson-PMF window convolutions (a->m->e->out) compose into
one 28-tap band filter C3n over s, plus warmup boundary terms (a rank-64
matmul wc = wfeat @ bm over the last-10-day warmup features).

Device pipeline (time-on-partitions, log domain):
  Host uploads x[slot, b] = lnK + ln(rt)/T in fp16, split into 3 tau-chunks
  of 128 slots; each chunk is prefixed by 2 fp16 rows (hi+lo split of the
  f64-exact chunk-start log-carry, so chunk cumsums need no cross-chunk
  matmuls and the seed ln(s0) costs no extra precision).
    PE : c = L @ x   (lower-tri fp16 matmul = per-chunk cumsum, f32 PSUM)
    ACT: s = exp(c)  (PSUM -> SBUF bf16)
    PE : out = s.T @ B3band  (+ wfeat.T @ bm warmup term), bf16, f32 PSUM
    DVE/Pool: PSUM -> SBUF bf16 copy;  SP: batched DMAs in/out.
Sharding: batch B=16384 split 8 ways, pure data parallel, no collectives.
Host does only input packing (ln, two partial sums, fp16/bf16 casts) and
output unpacking; all per-day compute runs on device.
"""

import numpy as np
from ml_dtypes import bfloat16

import concourse.bacc as bacc
import concourse.bass as bass
import concourse.mybir as mybir
import concourse.tile as tile
from concourse.bass_utils import run_bass_kernel_spmd

# Problem constants (fixed by the nn.Module definition)
J = 10
T_SERIAL = 5.8
B = 16384
FORECAST = 365
N_CORES = 8
B_SHARD = B // N_CORES          # 2048
N_TILES = B_SHARD // 128        # 16 scenario tiles per core
N_GRP = 2                       # tile groups (tri/exp fusion granularity)
TPG = N_TILES // N_GRP          # 8 tiles per group
GCOLS = 3 * TPG * 128           # 3072 x-columns per group
DAYS = ((1, 126), (127, 252), (253, 365))       # day span per tau-chunk
BAND_W = ((0, 160), (128, 288), (240, 365))     # band output col windows
F16 = mybir.dt.float16
BF16 = mybir.dt.bfloat16
F32 = mybir.dt.float32
U16 = mybir.dt.uint16
CST_COLS = 128 + sum(e - o for o, e in BAND_W)  # L + B0 + B1 + B2 = 573
WF_COLS = B_SHARD + 32                          # wfeatT + bm (30 used)


def _build_nc():
    nc = bacc.Bacc()
    xin_d = nc.dram_tensor("xin", [128, N_GRP * GCOLS], F16, kind="ExternalInput")
    cst_d = nc.dram_tensor("cst", [128, CST_COLS], U16, kind="ExternalInput")
    wf_d = nc.dram_tensor("wf", [64, WF_COLS], U16, kind="ExternalInput")
    out_d = nc.dram_tensor("outp", [128, N_TILES * FORECAST], BF16,
                           kind="ExternalOutput")
    Exp = mybir.ActivationFunctionType.Exp

    Copy = mybir.ActivationFunctionType.Copy
    with tile.TileContext(nc) as tc:
        with (
            tc.tile_pool(name="big", bufs=1) as big,
            tc.tile_pool(name="tri", bufs=2, space=bass.MemorySpace.PSUM) as tri_p,
            tc.tile_pool(name="band", bufs=2, space=bass.MemorySpace.PSUM) as band_p,
        ):
            # PE p-state warmup: keep PE busy from t~0 so the real matmuls
            # run at the ramped (2.4GHz) clock instead of mid/low p-state.
            scratch = big.tile([128, 256], F16, tag="scr")
            nc.gpsimd.memset(scratch[:], 0.0)
            wps = band_p.tile([128, 1024], F32, tag="band")
            for _ in range(14):
                nc.tensor.matmul(wps[:, 0:256], scratch[:, 0:128],
                                 scratch[:, 0:256], start=True, stop=True)

            # cst+wf ride the ACT HWDGE queue; x chunks stream on SP so the
            # transfer order is cst, x00, wf, x01, x02, x10, x11, x12.
            cst = big.tile([128, CST_COLS], U16, tag="cst")
            nc.scalar.dma_start(cst[:], cst_d[:])
            wf = big.tile([64, WF_COLS], U16, tag="wf")
            nc.scalar.dma_start(wf[:], wf_d[:])
            x_sb = big.tile([128, N_GRP * GCOLS], F16, tag="x")
            for g in range(N_GRP):
                for k in range(3):
                    c0 = g * GCOLS + k * 1024
                    nc.sync.dma_start(x_sb[:, c0:c0 + 1024], xin_d[:, c0:c0 + 1024])
            s_sb = big.tile([128, N_GRP * GCOLS], BF16, tag="s")
            o_sb = big.tile([128, N_TILES * FORECAST], BF16, tag="o")

            Lw = cst[:, 0:128].bitcast(F16)         # lower-tri ones (cumsum)
            bv, off = [], 128
            for o0, o1 in BAND_W:
                bv.append(cst[:, off:off + (o1 - o0)].bitcast(BF16))
                off += o1 - o0
            bm_v = wf[:, B_SHARD:B_SHARD + 30].bitcast(BF16)

            def pair_view(ap2d):
                # [128, 1024/730] -> [128, 2, 365] (strided / packed)
                return ap2d.rearrange("p (two c) -> p two c", two=2)

            for g in range(N_GRP):
                gb = g * GCOLS
                for k in range(3):
                    c0 = gb + k * 1024
                    cps = tri_p.tile([128, 1024], F32, tag="tri")
                    nc.tensor.matmul(cps[:, 0:512], Lw, x_sb[:, c0:c0 + 512],
                                     start=True, stop=True)
                    nc.tensor.matmul(cps[:, 512:1024], Lw,
                                     x_sb[:, c0 + 512:c0 + 1024],
                                     start=True, stop=True)
                    nc.scalar.activation(s_sb[:, c0:c0 + 1024], cps[:], Exp)
                for p in range(TPG // 2):           # tile pairs (2 PSUM banks)
                    pg = g * (TPG // 2) + p
                    ps = band_p.tile([128, 1024], F32, tag="band")
                    for h in range(2):              # tile h of the pair
                        gt = g * TPG + 2 * p + h
                        hb = 512 * h
                        for k in range(3):
                            o0, o1 = BAND_W[k]
                            sk = gb + k * 1024 + (2 * p + h) * 128
                            nc.tensor.matmul(ps[:, hb + o0:hb + o1],
                                             s_sb[:, sk:sk + 128], bv[k],
                                             start=(k == 0), stop=False)
                        nc.tensor.matmul(
                            ps[:, hb:hb + 30],
                            wf[:, gt * 128:(gt + 1) * 128].bitcast(BF16),
                            bm_v, start=False, stop=True)
                    oc = 2 * p * FORECAST + g * TPG * FORECAST
                    dst = pair_view(o_sb[:, oc:oc + 2 * FORECAST])
                    src = pair_view(ps[:, 0:1024])[:, :, 0:FORECAST]
                    if pg == 7:                     # decouple the very tail
                        nc.vector.tensor_copy(dst[:, 0:1, :], src[:, 0:1, :])
                        nc.scalar.activation(dst[:, 1:2, :], src[:, 1:2, :], Copy)
                        nc.sync.dma_start(out_d[:, oc:oc + FORECAST],
                                          o_sb[:, oc:oc + FORECAST])
                        nc.sync.dma_start(
                            out_d[:, oc + FORECAST:oc + 2 * FORECAST],
                            o_sb[:, oc + FORECAST:oc + 2 * FORECAST])
                    else:
                        if pg in (5, 6):            # ACT helps once exps drain
                            nc.scalar.activation(dst, src, Copy)
                        else:
                            nc.vector.tensor_copy(dst, src)
                        nc.sync.dma_start(out_d[:, oc:oc + 2 * FORECAST],
                                          o_sb[:, oc:oc + 2 * FORECAST])
    nc.compile()
    return nc


def _host_constants(eps, delta, rho_M, rho_X, rho_G, pi_M, pi_X, pi_G):
    """C3n band taps and the warmup-feature matrix bm (f64)."""
    K = delta[0] + eps * delta[1]
    C3 = np.zeros(3 * (J - 1) + 1)
    for v in range(2):
        W = np.convolve(np.convolve(pi_G[v], pi_X[v]), pi_M[v])
        C3 += rho_G[v] * rho_X[v] * rho_M[v] * delta[v] * W
    C3n = C3 / K

    bm = np.zeros((64, 30))
    for v in range(2):
        for D in range(10):
            tau = D - 9
            for t in range(1, 31):
                col = t - 1
                j = t - 1 - tau
                if 0 <= j <= 9:
                    bm[40 + 10 * v + D, col] += rho_G[v] * pi_G[v, j]
                acc = 0.0
                for jj in range(10):
                    k = t - 2 - jj - tau
                    if 0 <= k <= 9 and (t - 1 - jj) >= 1:
                        acc += pi_G[v, jj] * pi_X[v, k]
                bm[20 + 10 * v + D, col] += rho_G[v] * rho_X[v] * acc
                acc = 0.0
                for jj in range(10):
                    for k in range(10):
                        l = t - 3 - jj - k - tau
                        if (0 <= l <= 9 and (t - 1 - jj) >= 1
                                and (t - 2 - jj - k) >= 1):
                            acc += pi_G[v, jj] * pi_X[v, k] * pi_M[v, l]
                bm[10 * v + D, col] += rho_G[v] * rho_X[v] * rho_M[v] * acc
    return K, C3n, bm


_CACHE = {}


def _prep(inputs):
    r_t = np.asarray(inputs["r_t"], np.float64)
    wa = np.asarray(inputs["warmup_asymp"], np.float64)
    wm = np.asarray(inputs["warmup_mild"], np.float64)
    we = np.asarray(inputs["warmup_extreme"], np.float64)
    eps = float(np.asarray(inputs["eps"], np.float64)[0])
    delta, rho_M, rho_X, rho_G, pi_M, pi_X, pi_G = (
        np.asarray(inputs[k], np.float64)
        for k in ("delta", "rho_M", "rho_X", "rho_G", "pi_M", "pi_X", "pi_G"))

    K, C3n, bm = _host_constants(eps, delta, rho_M, rho_X, rho_G,
                                 pi_M, pi_X, pi_G)
    invT = 1.0 / T_SERIAL

    if "nc" not in _CACHE:
        _CACHE["nc"] = _build_nc()
    nc = _CACHE["nc"]

    # x rows and f64-exact chunk carries
    x = np.log(K) + invT * np.log(r_t)              # (B, 365)
    wfeat = np.zeros((B, 64))
    for ci, arr in enumerate((wa, wm, we)):
        for v in range(2):
            wfeat[:, 20 * ci + 10 * v: 20 * ci + 10 * v + 10] = arr[v, :, 20:30]
    s0 = wfeat[:, 9] + eps * wfeat[:, 19]
    carry = np.log(s0)
    Xp = np.zeros((3, 128, B), np.float16)          # [chunk, slot, b]
    for k, (d0, d1) in enumerate(DAYS):
        hi = carry.astype(np.float16)
        Xp[k, 0] = hi
        Xp[k, 1] = (carry - hi.astype(np.float64)).astype(np.float16)
        Xp[k, 2:2 + d1 - d0 + 1] = x[:, d0 - 1:d1].T.astype(np.float16)
        carry = carry + x[:, d0 - 1:d1].sum(axis=1)

    # band matrices: chunk slot p=2+i holds s(d0+i); coeff C3n[t-tau-3]
    Bc = np.zeros((3, 128, FORECAST))
    for k, (d0, d1) in enumerate(DAYS):
        for p in range(2, 2 + d1 - d0 + 1):
            tau = d0 + p - 2
            lo, hi_ = tau + 3, min(tau + 30, FORECAST)
            if lo <= hi_:
                Bc[k, p, lo - 1:hi_] = C3n[0:hi_ - lo + 1]

    cst = np.zeros((128, CST_COLS), np.uint16)
    cst[:, 0:128] = np.triu(np.ones((128, 128), np.float16)).view(np.uint16)
    off = 128
    for k, (o0, o1) in enumerate(BAND_W):
        cst[:, off:off + o1 - o0] = Bc[k][:, o0:o1].astype(bfloat16).view(np.uint16)
        off += o1 - o0

    wfT = wfeat.T.astype(bfloat16).view(np.uint16)   # (64, B)
    bm16 = bm.astype(bfloat16).view(np.uint16)       # (64, 30)

    in_maps = []
    for c in range(N_CORES):
        sl = slice(c * B_SHARD, (c + 1) * B_SHARD)
        # [k, p, g, t, b] -> [p, g, k, t, b]
        xc = np.ascontiguousarray(
            Xp[:, :, sl].reshape(3, 128, N_GRP, TPG, 128)
            .transpose(1, 2, 0, 3, 4).reshape(128, N_GRP * GCOLS))
        wfc = np.zeros((64, WF_COLS), np.uint16)
        wfc[:, 0:B_SHARD] = wfT[:, sl]
        wfc[:, B_SHARD:B_SHARD + 30] = bm16
        in_maps.append({"xin": xc, "cst": cst, "wf": wfc})
    return nc, in_maps


def kernel(**inputs):
    nc, in_maps = _prep(inputs)
    res = run_bass_kernel_spmd(nc, in_maps, list(range(N_CORES)))
    outs = []
    for c in range(N_CORES):
        o = np.asarray(res.results[c]["outp"]).astype(np.float32)
        outs.append(o.reshape(128, N_TILES, FORECAST)
                    .transpose(1, 0, 2).reshape(B_SHARD, FORECAST))
    return np.concatenate(outs, axis=0)

